# revision 1
# baseline (speedup 1.0000x reference)
"""Trainium2 Bass kernel for nn_AdaptiveTemp (adaptive temperature from
per-sample Jacobian Gram norms).

Math: for each sample x (D=3072), with logits l = xW+b, s = softmax(l),
p = alpha*s + eps_ns, nc = sqrt(p), the reference computes the Jacobian J of
y_m = 2*nc_m/(1-nc_9) wrt x, G = rho^2 * J J^T, and
temp = 2*arccos(sum(sqrt(p/10))) / (0.1 * max-abs-row-sum(G)).

Key identity: J = A W^T with A = dy/dl [9,10], so G = A K A^T with
K = W^T W (10x10, shared).  Further A[m,:] = nc_m*(g1 e_m + g2 e_9 - g3 p)
with per-sample scalars g1 = 1/(1-nc9), g2 = nc9/(1-nc9)^2, g3 = g1+g2
(dropping the 1e-7 floor inside A only, error ~1e-5).  Hence

    G[m,n] = nc_m nc_n (g1^2 K[m,n] + beta_m + beta_n),
    beta   = g1 g2 K[:9,9] - g1 g3 (K p)[:9] + c0/2,
    c0     = g2^2 K[9,9] - 2 g2 g3 (K p)[9] + g3^2 (p.K p).

arccos via the series arccos(1-e) = sqrt(2e)*(1 + e/12 + 3e^2/160 +
5e^3/896 + 35e^4/18432) (e = 1-z in [0.02, 0.36] here), and
nc = exp(0.5*l - 0.5*ln(sum exp(l)/alpha)) (logits are O(5): no max
subtraction needed), so the only ACT functions are Exp and Ln — one
table set (natural_log_exp_and_others).

Layout: per-core shard of 256 samples; x passed host-transposed and
chunk-blocked as [128, 24, 256] so the contraction lands on SBUF
partitions with per-partition-contiguous DMA. W cols and x cols sit
adjacent per chunk in one SBUF tile so a single f32r matmul per chunk
accumulates both K = W^T W (cols 0:10) and logits^T (cols 10:266).
"""

import numpy as np
from contextlib import ExitStack

import concourse.bass as bass
import concourse.bacc as bacc
import concourse.tile as tile
from concourse import mybir
from concourse.masks import make_identity

f32 = mybir.dt.float32
f16 = mybir.dt.float16
AF = mybir.ActivationFunctionType
OP = mybir.AluOpType
X = mybir.AxisListType.X

NCORES = 8
B = 2048
BL = B // NCORES          # 256 samples per core
D = 3072
C = 10
M = 9
P = 128
ND = D // P               # 24 contraction chunks
NG = BL // P              # 2 sample groups of 128
NDMA = 8                  # x load split (chunks per DMA = ND // NDMA)
W_CHUNK = C + BL          # 266 = [w cols | x cols] per chunk
ALPHA = 1.0 - C * 1e-7
ISQ10 = 1.0 / float(np.sqrt(10.0))
# arccos series coeffs, folded with 2/EPSILON = 20
PC = [20.0 * c for c in (1.0, 1.0 / 12, 3.0 / 160, 5.0 / 896, 35.0 / 18432)]


def _v(t, dims, off=0):
    """Free-dim view of an SBUF tile AP, keeping its partition dim."""
    return bass.AP(
        tensor=t.tensor,
        offset=t.offset + off,
        ap=[list(t.ap[0])] + [list(d) for d in dims],
    )


def _patch_act_tables():
    """Force Exp/Ln to resolve to the one table set containing both, so the
    ACT table is loaded exactly once (set order in act_info.json otherwise
    makes the chooser thrash exp_and_others <-> natural_log)."""
    import concourse.hw_specs as hw_specs
    import concourse.bacc as bacc_mod
    if getattr(hw_specs, "_adaptive_temp_patched", False):
        return
    orig = hw_specs.get_activation_tables

    def patched(arch):
        t = orig(arch)
        for name, fns in t.items():
            if name != "natural_log_exp_and_others":
                fns.discard(AF.Exp)
                fns.discard(AF.Ln)
        return t

    hw_specs.get_activation_tables = patched
    hw_specs._adaptive_temp_patched = True
    for mod in (bacc_mod,):
        if hasattr(mod, "get_activation_tables"):
            mod.get_activation_tables = patched


def build_bass():
    _patch_act_tables()
    nc = bacc.Bacc("TRN2", target_bir_lowering=False, debug=False)
    # xh[k, i, s] = x[sample s, feature i*128+k]  (host pre-transposed)
    xh = nc.dram_tensor("xh", [P, ND, BL], f16, kind="ExternalInput").ap()
    # wh[k, i, c] = W[i*128+k, c]  (host pre-blocked, partition-contiguous)
    wh = nc.dram_tensor("wh", [P, ND, C], f16, kind="ExternalInput").ap()
    bt = nc.dram_tensor("bt", [C, 1], f32, kind="ExternalInput").ap()
    out = nc.dram_tensor("out", [BL, 1], f32, kind="ExternalOutput").ap()
    ksc = nc.dram_tensor("kscratch", [C, C], f32, kind="Internal").ap()

    with tile.TileContext(nc) as tc, ExitStack() as ctx:
        const = ctx.enter_context(tc.tile_pool(name="const", bufs=1))
        ps = ctx.enter_context(tc.tile_pool(name="ps", bufs=1, space="PSUM"))
        wk = ctx.enter_context(tc.tile_pool(name="wk", bufs=1))

        # --- ACT table warmup: pull the (single) exp/ln table load to t=0 ---
        dum = const.tile([1, 1], f32, name="dum")
        nc.vector.memset(dum, 1.0)
        nc.scalar.activation(dum, dum, AF.Ln)
        nc.scalar.activation(dum, dum, AF.Exp)

        # --- identity for PE transposes (also sliced [10,10]) ---
        ident = const.tile([P, P], f32, name="ident")
        make_identity(nc, ident)

        # --- constant loads: FIRST on the HWDGE ring (before the x loads)
        #     so their DMA-lane ticks come first and the K matmuls' wait
        #     fires as soon as W lands.  (SWDGE would get starved behind
        #     the HWDGE x stream -- measured 12us of delay.) ---
        wt = const.tile([P, ND, C], f16, name="wt")
        nc.sync.dma_start(out=wt, in_=wh)
        btile = const.tile([C, 1], f32, name="btile")
        nc.sync.dma_start(out=btile, in_=bt)

        # --- x loads: big per-partition-contiguous DMAs on both HWDGE
        #     rings (sync + scalar); last groups small so the final
        #     completion receipt exposes less tail latency ---
        xall = const.tile([P, ND, BL], f16, name="xall")
        x_groups = [5, 5, 4, 4, 3, 2, 1]
        assert sum(x_groups) == ND
        pos = 0
        for j, sz in enumerate(x_groups):
            eng = nc.sync if j % 2 == 0 else nc.scalar
            eng.dma_start(out=xall[:, pos:pos + sz, :],
                          in_=xh[:, pos:pos + sz, :])
            pos += sz

        # --- K = W^T W first: runs while x is still streaming, so the
        #     K DRAM-broadcast roundtrip is off the critical path ---
        kps = ps.tile([C, C], f32, name="kps")
        for i in range(ND):
            nc.tensor.matmul(kps, lhsT=wt[:, i, :], rhs=wt[:, i, :],
                             start=(i == 0), stop=(i == ND - 1))
        ksb = wk.tile([C, C], f32, name="ksb")
        nc.scalar.copy(ksb, kps)
        nc.gpsimd.dma_start(out=ksc, in_=ksb)
        kb = const.tile([P, C, C], f32, name="kb")
        nc.gpsimd.dma_start(
            out=kb,
            in_=bass.AP(tensor=ksc.tensor, offset=0, ap=[[0, P], [C, C], [1, C]]),
        )

        # --- logits^T accumulation (fp16 operands, N=256 full rate) ---
        ltp = ps.tile([C, BL], f32, name="ltp")
        for i in range(ND):
            nc.tensor.matmul(ltp, lhsT=wt[:, i, :], rhs=xall[:, i, :],
                             start=(i == 0), stop=(i == ND - 1))

        # --- logits + b, transpose to sample-major [128, 2, 10] psum ---
        lts = wk.tile([C, BL], f32, name="lts")
        ltg = ps.tile([P, NG, C], f32, name="ltg")
        for g in range(NG):
            nc.scalar.activation(out=lts[:, g * P:(g + 1) * P],
                                 in_=ltp[:, g * P:(g + 1) * P],
                                 func=AF.Identity, bias=btile, scale=1.0)
            nc.tensor.transpose(ltg[:, g, :], lts[:, g * P:(g + 1) * P],
                                ident[0:C, 0:C])

        # --- nc = exp(0.5*l)*(alpha/S)^0.5, S = sum exp(l) (both groups
        #     batched; sums on DVE, no ACT accumulator reads) ---
        expd = wk.tile([P, NG, C], f32, name="expd")
        nc.scalar.activation(out=expd, in_=ltg, func=AF.Exp, scale=1.0)
        e05 = wk.tile([P, NG, C], f32, name="e05")
        nc.scalar.activation(out=e05, in_=ltg, func=AF.Exp, scale=0.5)
        sumexp = wk.tile([P, NG], f32, name="sumexp")
        nc.vector.tensor_reduce(out=sumexp, in_=expd, axis=X, op=OP.add)
        lnS = wk.tile([P, NG], f32, name="lnS")
        nc.scalar.activation(out=lnS, in_=sumexp, func=AF.Ln, scale=1.0 / ALPHA)
        rsqS = wk.tile([P, NG], f32, name="rsqS")
        nc.scalar.activation(out=rsqS, in_=lnS, func=AF.Exp, scale=-0.5)
        ncv = wk.tile([P, NG, C], f32, name="ncv")
        rsqSe = _v(rsqS, [[1, NG], [0, C]])
        nc.vector.tensor_tensor(out=ncv, in0=e05, in1=rsqSe, op=OP.mult)
        sumnc = wk.tile([P, NG], f32, name="sumnc")
        nc.vector.tensor_reduce(out=sumnc, in_=ncv, axis=X, op=OP.add)
        pt = wk.tile([P, NG, C], f32, name="pt")
        nc.vector.tensor_mul(pt, ncv, ncv)          # p = nc^2

        # --- h = K p  (per-sample 10-vector), rq = p . h ---
        tmp2 = wk.tile([P, NG, C, C], f32, name="tmp2")
        kb4 = _v(kb, [[0, NG], [C, C], [1, C]])
        pt4 = _v(pt, [[C, NG], [0, C], [1, C]])
        nc.vector.tensor_tensor(out=tmp2, in0=kb4, in1=pt4, op=OP.mult)
        h = wk.tile([P, NG, C], f32, name="h")
        nc.vector.tensor_reduce(out=h, in_=tmp2, axis=X, op=OP.add)
        rq = wk.tile([P, NG], f32, name="rq")
        scr = wk.tile([P, NG, C], f32, name="scr")
        nc.vector.tensor_mul(scr, pt, h)
        nc.vector.tensor_reduce(out=rq, in_=scr, axis=X, op=OP.add)

        # --- per-sample scalars.  With w1 = nc9/(1-nc9), w2 = 1+w1, every
        # g-term factors as g_i = g1^i * (w-term), and the overall g1^2 on
        # G cancels against rho^2 = (1-nc9)^2 in the final quotient, so g1
        # never multiplies the G chain at all. ---
        ncM = _v(ncv, [[C, NG]], off=M)             # nc_9 per group  [P, 2]
        hM = _v(h, [[C, NG]], off=M)                # h_9             [P, 2]
        r1 = wk.tile([P, NG], f32, name="r1")
        nc.vector.tensor_scalar(out=r1, in0=ncM, scalar1=-1.0, scalar2=1.0,
                                op0=OP.mult, op1=OP.add)    # 1 - nc9
        g1 = wk.tile([P, NG], f32, name="g1")
        nc.vector.reciprocal(g1, r1)
        w1 = wk.tile([P, NG], f32, name="w1")
        nc.vector.tensor_mul(w1, ncM, g1)           # nc9/(1-nc9)
        w2 = wk.tile([P, NG], f32, name="w2")
        nc.vector.tensor_scalar_add(w2, w1, 1.0)    # 1 + w1
        w1sq = wk.tile([P, NG], f32, name="w1sq")
        nc.vector.tensor_mul(w1sq, w1, w1)
        w2sq = wk.tile([P, NG], f32, name="w2sq")
        nc.vector.tensor_mul(w2sq, w2, w2)
        w12 = wk.tile([P, NG], f32, name="w12")
        nc.vector.tensor_mul(w12, w1, w2)

        # c0~ = w1^2 K99 - 2 w1w2 h9 + w2^2 rq ;  then /2
        kmm = _v(kb, [[0, NG]], off=M * C + M)      # K[9,9] bcast    [P, 2]
        a2 = wk.tile([P, NG], f32, name="a2")
        nc.vector.tensor_mul(a2, w1sq, kmm)
        b2 = wk.tile([P, NG], f32, name="b2")
        nc.vector.tensor_mul(b2, w12, hM)
        c2t = wk.tile([P, NG], f32, name="c2t")
        nc.vector.tensor_mul(c2t, w2sq, rq)
        c0 = wk.tile([P, NG], f32, name="c0")
        nc.vector.tensor_scalar(out=c0, in0=b2, scalar1=-2.0, scalar2=None,
                                op0=OP.mult)
        nc.vector.tensor_add(c0, c0, a2)
        nc.vector.tensor_add(c0, c0, c2t)
        nc.vector.tensor_scalar_mul(c0, c0, 0.5)

        # --- beta~ = w1*K[:9,9] - w2*h[:9] + c0~/2   [P, 2, 9] ---
        beta = wk.tile([P, NG, M], f32, name="beta")
        km9 = _v(kb, [[0, NG], [C, M]], off=M)      # K[m,9]
        w1e = _v(w1, [[1, NG], [0, M]])
        nc.vector.tensor_tensor(out=beta, in0=km9, in1=w1e, op=OP.mult)
        yg = wk.tile([P, NG, M], f32, name="yg")
        h9 = _v(h, [[C, NG], [1, M]])
        w2e = _v(w2, [[1, NG], [0, M]])
        nc.vector.tensor_tensor(out=yg, in0=h9, in1=w2e, op=OP.mult)
        nc.vector.tensor_sub(beta, beta, yg)
        c0e = _v(c0, [[1, NG], [0, M]])
        nc.vector.tensor_tensor(out=beta, in0=beta, in1=c0e, op=OP.add)

        # --- Gbar~ = K[:9,:9] + beta~_m + beta~_n ; weighted row sums ---
        gt = wk.tile([P, NG, M, M], f32, name="gt")
        kf4 = _v(kb, [[0, NG], [C, M], [1, M]])
        beta_rep = _v(beta, [[M, NG], [1, M], [0, M]])
        nc.vector.tensor_tensor(out=gt, in0=kf4, in1=beta_rep, op=OP.add)
        beta_til = _v(beta, [[M, NG], [0, M], [1, M]])
        nc.vector.tensor_tensor(out=gt, in0=gt, in1=beta_til, op=OP.add)
        nc9t = _v(ncv, [[C, NG], [0, M], [1, M]])
        nc.vector.tensor_tensor(out=gt, in0=gt, in1=nc9t, op=OP.mult)
        rs = wk.tile([P, NG, M], f32, name="rs")
        nc.vector.tensor_reduce(out=rs, in_=gt, axis=X, op=OP.add,
                                apply_absolute_value=True)
        mx = wk.tile([P, NG], f32, name="mx")
        scr9 = wk.tile([P, NG, M], f32, name="scr9")
        nc9v = _v(ncv, [[C, NG], [1, M]])
        nc.vector.tensor_mul(scr9, rs, nc9v)
        nc.vector.tensor_reduce(out=mx, in_=scr9, axis=X, op=OP.max)

        # --- delta series (gpsimd: runs parallel to the DVE G chain) ---
        e2 = wk.tile([P, NG], f32, name="e2")
        nc.gpsimd.tensor_scalar(out=e2, in0=sumnc, scalar1=-ISQ10,
                                scalar2=1.0, op0=OP.mult, op1=OP.add)
        ln2e = wk.tile([P, NG], f32, name="ln2e")
        nc.scalar.activation(out=ln2e, in_=e2, func=AF.Ln, scale=2.0)
        sq2e = wk.tile([P, NG], f32, name="sq2e")
        nc.scalar.activation(out=sq2e, in_=ln2e, func=AF.Exp, scale=0.5)
        pol = wk.tile([P, NG], f32, name="pol")
        nc.gpsimd.tensor_scalar(out=pol, in0=e2, scalar1=PC[4], scalar2=PC[3],
                                op0=OP.mult, op1=OP.add)
        for k in (2, 1, 0):
            nc.gpsimd.tensor_mul(pol, pol, e2)
            nc.gpsimd.tensor_scalar_add(pol, pol, PC[k])
        num = wk.tile([P, NG], f32, name="num")
        nc.gpsimd.tensor_mul(num, sq2e, pol)
        # temp = num / mx~  (the g1^2 on G cancelled rho^2 exactly)
        rmx = wk.tile([P, NG], f32, name="rmx")
        nc.vector.reciprocal(rmx, mx)
        tempv = wk.tile([P, NG], f32, name="tempv")
        nc.vector.tensor_mul(tempv, num, rmx)

        # --- transpose [128, 2] -> [2, 128] so the output DMA is 2
        #     contiguous 512B descriptors instead of 256 x 4B ---
        otp = ps.tile([NG, P], f32, name="otp")
        nc.tensor.transpose(otp, tempv, ident)
        osb = wk.tile([NG, P], f32, name="osb")
        nc.vector.tensor_copy(osb, otp)
        nc.sync.dma_start(out=out.rearrange("(g p) o -> g (p o)", g=NG),
                          in_=osb)
    nc.compile()
    return nc


_NC_CACHE = None


def _get_nc():
    global _NC_CACHE
    if _NC_CACHE is None:
        _NC_CACHE = build_bass()
    return _NC_CACHE


def make_in_maps(data: np.ndarray, W: np.ndarray, b: np.ndarray):
    x = np.asarray(data, dtype=np.float32).reshape(B, D)
    Wf = np.ascontiguousarray(np.asarray(W, dtype=np.float32))
    btf = np.ascontiguousarray(np.asarray(b, dtype=np.float32).reshape(C, 1))
    whp = np.ascontiguousarray(
        Wf.reshape(ND, P, C).transpose(1, 0, 2).astype(np.float16))
    in_maps = []
    for i in range(NCORES):
        shard = x[i * BL:(i + 1) * BL, :]           # [256, 3072]
        xhp = np.ascontiguousarray(
            shard.T.reshape(ND, P, BL).transpose(1, 0, 2).astype(np.float16))
        in_maps.append({"xh": xhp, "wh": whp, "bt": btf})
    return in_maps


def kernel(data: np.ndarray, W: np.ndarray, b: np.ndarray) -> np.ndarray:
    from concourse.bass_utils import run_bass_kernel_spmd

    in_maps = make_in_maps(data, W, b)
    nc = _get_nc()
    res = run_bass_kernel_spmd(nc, in_maps, core_ids=list(range(NCORES)))
    outs = [res.results[i]["out"] for i in range(NCORES)]
    return np.concatenate(outs, axis=0).astype(np.float32)



# revision 2
# speedup vs baseline: 1.0590x; 1.0590x over previous
"""Trainium2 Bass kernel for nn_AdaptiveTemp (adaptive temperature from
per-sample Jacobian Gram norms).

Math: for each sample x (D=3072), with logits l = xW+b, s = softmax(l),
p = alpha*s (the 1e-7 floor is dropped: ~1e-5 error), nc = sqrt(p), the
reference computes the Jacobian J of y_m = 2*nc_m/(1-nc_9) wrt x,
G = rho^2 * J J^T, and temp = 2*arccos(sum(sqrt(p/10))) /
(0.1 * max-abs-row-sum(G)).

Identity used here (g1^2 on G cancels rho^2 exactly): with
q = (p - nc9*e9)/(nc9 - 1), r = K q (K = W^T W, 10x10 shared),
c0 = q.K q:

    Gbar[m,n] = nc_m nc_n (K[m,n] + r_m + r_n + c0),   m,n < 9
    temp = 20*arccos(sum nc/sqrt(10)) / max_m sum_n |Gbar[m,n]|

arccos via the series arccos(1-e) = sqrt(2e)*(1 + e/12 + 3e^2/160 +
5e^3/896 + 35e^4/18432) (e in [0.02, 0.36] here), and
nc = exp(0.5*l - 0.5*ln(sum exp(l)/alpha)), so the only ACT table
functions are Exp and Ln — one table set (natural_log_exp_and_others).

Layout: per-core shard of 256 samples; x host-transposed and blocked as
[128, 24, 256] so the contraction lands on SBUF partitions with
per-partition-contiguous DMA.  DMA issues come FIRST (before identity /
act warmup) on both HWDGE rings; W is split across the two rings so the
K = W^T W chain starts as early as possible, and x is loaded in 4
groups per ring (separate tiles -> precise per-group completion events,
the PE logits chain chases the DMA stream in arrival order; PSUM
accumulation order is commutative).  No SWDGE (gpsimd) DMAs: the K
DRAM-broadcast roundtrip rides the sync HWDGE ring, which also avoids
the Pool DGE-init memsets at kernel start.
"""

import numpy as np
from contextlib import ExitStack

import concourse.bass as bass
import concourse.bacc as bacc
import concourse.tile as tile
from concourse import mybir
from concourse.masks import make_identity

f32 = mybir.dt.float32
f16 = mybir.dt.float16
AF = mybir.ActivationFunctionType
OP = mybir.AluOpType
X = mybir.AxisListType.X

NCORES = 8
B = 2048
BL = B // NCORES          # 256 samples per core
D = 3072
C = 10
M = C - 1
P = 128
ND = D // P               # 24 contraction chunks
NG = BL // P              # 2 sample groups of 128
ALPHA = 1.0 - C * 1e-7
ISQ10 = 1.0 / float(np.sqrt(10.0))
# arccos series coeffs, folded with 2/EPSILON = 20
PC = [20.0 * c for c in (1.0, 1.0 / 12, 3.0 / 160, 5.0 / 896, 35.0 / 18432)]

# x chunk ranges per HWDGE ring (sync gets 0:12, scalar 12:24), small
# groups first so the PE chain starts early, larger later (descriptor
# overhead ~110ns/partition-line makes many small groups slow overall).
SYNC_CH = [(0, 2), (2, 5), (5, 8), (8, 12)]
SCAL_CH = [(12, 14), (14, 17), (17, 20), (20, 24)]
# PE consumption order: interleave rings by expected arrival.
MM_ORDER = [0, 4, 1, 5, 2, 6, 3, 7]   # index into SYNC_CH + SCAL_CH


def _v(t, dims, off=0):
    """Free-dim view of an SBUF tile AP, keeping its partition dim."""
    return bass.AP(
        tensor=t.tensor,
        offset=t.offset + off,
        ap=[list(t.ap[0])] + [list(d) for d in dims],
    )


def _patch_act_tables():
    """Force Exp/Ln to resolve to the one table set containing both, so the
    ACT table is loaded exactly once."""
    import concourse.hw_specs as hw_specs
    import concourse.bacc as bacc_mod
    if getattr(hw_specs, "_adaptive_temp_patched", False):
        return
    orig = hw_specs.get_activation_tables

    def patched(arch):
        t = orig(arch)
        for name, fns in t.items():
            if name != "natural_log_exp_and_others":
                fns.discard(AF.Exp)
                fns.discard(AF.Ln)
        return t

    hw_specs.get_activation_tables = patched
    hw_specs._adaptive_temp_patched = True
    for mod in (bacc_mod,):
        if hasattr(mod, "get_activation_tables"):
            mod.get_activation_tables = patched


def build_bass():
    _patch_act_tables()
    nc = bacc.Bacc("TRN2", target_bir_lowering=False, debug=False)
    # xh[k, i, s] = x[sample s, feature i*128+k]  (host pre-transposed)
    xh = nc.dram_tensor("xh", [P, ND, BL], f16, kind="ExternalInput").ap()
    # wh[k, i, c] = W[i*128+k, c]
    wh = nc.dram_tensor("wh", [P, ND, C], f16, kind="ExternalInput").ap()
    bt = nc.dram_tensor("bt", [C, 1], f32, kind="ExternalInput").ap()
    out = nc.dram_tensor("out", [BL, 1], f32, kind="ExternalOutput").ap()
    ksc = nc.dram_tensor("kscratch", [C, C], f32, kind="Internal").ap()

    with tile.TileContext(nc) as tc, ExitStack() as ctx:
        const = ctx.enter_context(tc.tile_pool(name="const", bufs=1))
        ps = ctx.enter_context(tc.tile_pool(name="ps", bufs=1, space="PSUM"))
        wk = ctx.enter_context(tc.tile_pool(name="wk", bufs=1))

        # ---- DMA issues first: W halves on both rings, then x groups ----
        wt = const.tile([P, ND, C], f16, name="wt")
        nc.sync.dma_start(out=wt[:, 0:12, :], in_=wh[:, 0:12, :])
        nc.scalar.dma_start(out=wt[:, 12:ND, :], in_=wh[:, 12:ND, :])
        btile = const.tile([C, 1], f32, name="btile")
        nc.scalar.dma_start(out=btile, in_=bt)

        xg = []
        for lo, hi in SYNC_CH:
            t = const.tile([P, hi - lo, BL], f16, name=f"xs{lo}")
            nc.sync.dma_start(out=t, in_=xh[:, lo:hi, :])
            xg.append((t, lo, hi))
        for lo, hi in SCAL_CH:
            t = const.tile([P, hi - lo, BL], f16, name=f"xa{lo}")
            nc.scalar.dma_start(out=t, in_=xh[:, lo:hi, :])
            xg.append((t, lo, hi))

        # ---- ACT table warmup (single natural_log_exp_and_others load) ----
        dum = const.tile([1, 1], f32, name="dum")
        nc.vector.memset(dum, 1.0)
        nc.scalar.activation(dum, dum, AF.Ln)
        nc.scalar.activation(dum, dum, AF.Exp)

        # ---- identity for PE transposes ----
        ident = const.tile([P, P], f32, name="ident")
        make_identity(nc, ident)

        # ---- K = W^T W (starts as soon as wt lands; x still streaming) ----
        kps = ps.tile([C, C], f32, name="kps")
        for i in range(ND):
            nc.tensor.matmul(kps, lhsT=wt[:, i, :], rhs=wt[:, i, :],
                             start=(i == 0), stop=(i == ND - 1))
        ksb = wk.tile([C, C], f32, name="ksb")
        nc.scalar.copy(ksb, kps)
        # broadcast K to all 128 partitions via a DRAM roundtrip (HWDGE)
        nc.sync.dma_start(out=ksc, in_=ksb)
        kb = const.tile([P, C, C], f32, name="kb")
        nc.sync.dma_start(
            out=kb,
            in_=bass.AP(tensor=ksc.tensor, offset=0, ap=[[0, P], [C, C], [1, C]]),
        )

        # 0.5*b for the exp(l/2) bias (b is tiny; Copy needs no table)
        bt05 = wk.tile([C, 1], f32, name="bt05")
        nc.scalar.mul(bt05, btile, 0.5)

        # ---- logits^T accumulation, chasing the DMA stream ----
        ltp = ps.tile([C, BL], f32, name="ltp")
        first = True
        n_done = 0
        for gi in MM_ORDER:
            t, lo, hi = xg[gi]
            for i in range(hi - lo):
                n_done += 1
                nc.tensor.matmul(ltp, lhsT=wt[:, lo + i, :], rhs=t[:, i, :],
                                 start=first, stop=(n_done == ND))
                first = False

        # ---- exp straight out of PSUM (bias folded), PE transposes ----
        et = wk.tile([C, BL], f32, name="et")
        nc.scalar.activation(out=et, in_=ltp, func=AF.Exp, bias=btile,
                             scale=1.0)
        e05 = wk.tile([C, BL], f32, name="e05")
        nc.scalar.activation(out=e05, in_=ltp, func=AF.Exp, bias=bt05,
                             scale=0.5)
        ug = ps.tile([P, NG, C], f32, name="ug")    # exp(l) sample-major
        vg = ps.tile([P, NG, C], f32, name="vg")    # exp(l/2) sample-major
        for g in range(NG):
            nc.tensor.transpose(ug[:, g, :], et[:, g * P:(g + 1) * P],
                                ident[0:C, 0:C])
        for g in range(NG):
            nc.tensor.transpose(vg[:, g, :], e05[:, g * P:(g + 1) * P],
                                ident[0:C, 0:C])

        # ---- nc = exp(l/2) * (alpha/S)^0.5 ----
        sumexp = wk.tile([P, NG], f32, name="sumexp")
        nc.vector.tensor_reduce(out=sumexp, in_=ug, axis=X, op=OP.add)
        lnS = wk.tile([P, NG], f32, name="lnS")
        nc.scalar.activation(out=lnS, in_=sumexp, func=AF.Ln, scale=1.0 / ALPHA)
        rsqS = wk.tile([P, NG], f32, name="rsqS")
        nc.scalar.activation(out=rsqS, in_=lnS, func=AF.Exp, scale=-0.5)
        ncv = wk.tile([P, NG, C], f32, name="ncv")
        rsqSe = _v(rsqS, [[1, NG], [0, C]])
        nc.vector.tensor_tensor(out=ncv, in0=vg, in1=rsqSe, op=OP.mult)
        sumnc = wk.tile([P, NG], f32, name="sumnc")
        nc.vector.tensor_reduce(out=sumnc, in_=ncv, axis=X, op=OP.add)

        ncM = _v(ncv, [[C, NG]], off=M)             # nc_9 per group  [P, 2]

        # ---- delta series (gpsimd + scalar: parallel to the DVE chain) ----
        e2 = wk.tile([P, NG], f32, name="e2")
        nc.gpsimd.tensor_scalar(out=e2, in0=sumnc, scalar1=-ISQ10,
                                scalar2=1.0, op0=OP.mult, op1=OP.add)
        ln2e = wk.tile([P, NG], f32, name="ln2e")
        nc.scalar.activation(out=ln2e, in_=e2, func=AF.Ln, scale=2.0)
        sq2e = wk.tile([P, NG], f32, name="sq2e")
        nc.scalar.activation(out=sq2e, in_=ln2e, func=AF.Exp, scale=0.5)
        pol = wk.tile([P, NG], f32, name="pol")
        nc.gpsimd.tensor_scalar(out=pol, in0=e2, scalar1=PC[4], scalar2=PC[3],
                                op0=OP.mult, op1=OP.add)
        for k in (2, 1, 0):
            nc.gpsimd.tensor_mul(pol, pol, e2)
            nc.gpsimd.tensor_scalar_add(pol, pol, PC[k])
        num = wk.tile([P, NG], f32, name="num")
        nc.gpsimd.tensor_mul(num, sq2e, pol)

        # outer_{mn} = nc_m nc_n (gpsimd, off the DVE critical path)
        outer = wk.tile([P, NG, M, M], f32, name="outer")
        ncm_r = _v(ncv, [[C, NG], [1, M], [0, M]])
        ncm_c = _v(ncv, [[C, NG], [0, M], [1, M]])
        nc.gpsimd.tensor_tensor(out=outer, in0=ncm_r, in1=ncm_c, op=OP.mult)

        # ---- q = (p - nc9 e9)/(nc9 - 1): pt = nc^2 with col9 patched ----
        pt = wk.tile([P, NG, C], f32, name="pt")
        nc.gpsimd.tensor_mul(pt, ncv, ncv)          # p = nc^2
        r1n = wk.tile([P, NG], f32, name="r1n")
        nc.vector.tensor_scalar(out=r1n, in0=ncM, scalar1=1.0, scalar2=None,
                                op0=OP.subtract)    # nc9 - 1
        g1n = wk.tile([P, NG], f32, name="g1n")
        nc.vector.reciprocal(g1n, r1n)
        ptM = _v(pt, [[C, NG]], off=M)
        nc.gpsimd.tensor_tensor(out=ptM, in0=ncM, in1=r1n, op=OP.mult)
        q = wk.tile([P, NG, C], f32, name="q")
        g1ne = _v(g1n, [[1, NG], [0, C]])
        nc.vector.tensor_tensor(out=q, in0=pt, in1=g1ne, op=OP.mult)

        # ---- r = K q, c0 = q . r ----
        tmp = wk.tile([P, NG, C, C], f32, name="tmp")
        kb4 = _v(kb, [[0, NG], [C, C], [1, C]])
        q4 = _v(q, [[C, NG], [0, C], [1, C]])
        nc.vector.tensor_tensor(out=tmp, in0=kb4, in1=q4, op=OP.mult)
        r = wk.tile([P, NG, C], f32, name="r")
        nc.vector.tensor_reduce(out=r, in_=tmp, axis=X, op=OP.add)
        scr = wk.tile([P, NG, C], f32, name="scr")
        nc.vector.tensor_mul(scr, q, r)
        c0 = wk.tile([P, NG], f32, name="c0")
        nc.vector.tensor_reduce(out=c0, in_=scr, axis=X, op=OP.add)

        # ---- Gbar = (K[:9,:9] + r_m + r_n + c0) * outer; weighted norm ----
        gt = wk.tile([P, NG, M, M], f32, name="gt")
        r_rep = _v(r, [[C, NG], [1, M], [0, M]])
        r_til = _v(r, [[C, NG], [0, M], [1, M]])
        nc.vector.tensor_tensor(out=gt, in0=r_rep, in1=r_til, op=OP.add)
        kf4 = _v(kb, [[0, NG], [C, M], [1, M]])
        nc.vector.tensor_tensor(out=gt, in0=gt, in1=kf4, op=OP.add)
        c0e = _v(c0, [[1, NG], [0, M], [0, M]])
        nc.vector.tensor_tensor(out=gt, in0=gt, in1=c0e, op=OP.add)
        nc.vector.tensor_tensor(out=gt, in0=gt, in1=outer, op=OP.mult)
        rs = wk.tile([P, NG, M], f32, name="rs")
        nc.vector.tensor_reduce(out=rs, in_=gt, axis=X, op=OP.add,
                                apply_absolute_value=True)
        mx = wk.tile([P, NG], f32, name="mx")
        nc.vector.tensor_reduce(out=mx, in_=rs, axis=X, op=OP.max)
        rmx = wk.tile([P, NG], f32, name="rmx")
        nc.vector.reciprocal(rmx, mx)
        tempv = wk.tile([P, NG], f32, name="tempv")
        nc.vector.tensor_mul(tempv, num, rmx)

        # ---- transpose [128, 2] -> [2, 128]: contiguous output DMA ----
        otp = ps.tile([NG, P], f32, name="otp")
        nc.tensor.transpose(otp, tempv, ident)
        osb = wk.tile([NG, P], f32, name="osb")
        nc.vector.tensor_copy(osb, otp)
        nc.sync.dma_start(out=out.rearrange("(g p) o -> g (p o)", g=NG),
                          in_=osb)
    nc.compile()
    return nc


_NC_CACHE = None


def _get_nc():
    global _NC_CACHE
    if _NC_CACHE is None:
        _NC_CACHE = build_bass()
    return _NC_CACHE


def make_in_maps(data: np.ndarray, W: np.ndarray, b: np.ndarray):
    x = np.asarray(data, dtype=np.float32).reshape(B, D)
    Wf = np.ascontiguousarray(np.asarray(W, dtype=np.float32))
    btf = np.ascontiguousarray(np.asarray(b, dtype=np.float32).reshape(C, 1))
    whp = np.ascontiguousarray(
        Wf.reshape(ND, P, C).transpose(1, 0, 2).astype(np.float16))
    in_maps = []
    for i in range(NCORES):
        shard = x[i * BL:(i + 1) * BL, :]           # [256, 3072]
        xhp = np.ascontiguousarray(
            shard.T.reshape(ND, P, BL).transpose(1, 0, 2).astype(np.float16))
        in_maps.append({"xh": xhp, "wh": whp, "bt": btf})
    return in_maps


def kernel(data: np.ndarray, W: np.ndarray, b: np.ndarray) -> np.ndarray:
    from concourse.bass_utils import run_bass_kernel_spmd

    in_maps = make_in_maps(data, W, b)
    nc = _get_nc()
    res = run_bass_kernel_spmd(nc, in_maps, core_ids=list(range(NCORES)))
    outs = [res.results[i]["out"] for i in range(NCORES)]
    return np.concatenate(outs, axis=0).astype(np.float32)


# revision 6
# speedup vs baseline: 1.0960x; 1.0349x over previous
"""Trainium2 Bass kernel for nn_AdaptiveTemp (adaptive temperature from
per-sample Jacobian Gram norms).

Math: for each sample x (D=3072), with logits l = xW+b, s = softmax(l),
p = alpha*s (the 1e-7 floor is dropped: ~1e-5 error), nc = sqrt(p), the
reference computes the Jacobian J of y_m = 2*nc_m/(1-nc_9) wrt x,
G = rho^2 * J J^T, and temp = 2*arccos(sum(sqrt(p/10))) /
(0.1 * max-abs-row-sum(G)).

Identity used here (g1^2 on G cancels rho^2 exactly): with
q = (p - nc9*e9)/(nc9 - 1), r = K q (K = W^T W, 10x10 shared),
c0 = q.K q:

    Gbar[m,n] = nc_m nc_n (K[m,n] + r_m + r_n + c0),   m,n < 9
    temp = 20*arccos(sum nc/sqrt(10)) / max_m sum_n |Gbar[m,n]|

arccos via the series arccos(1-e) = sqrt(2e)*(1 + e/12 + 3e^2/160 +
5e^3/896 + 35e^4/18432) (e in [0.02, 0.36] here), and
nc = exp(0.5*l - 0.5*ln(sum exp(l)/alpha)), so the only ACT table
functions are Exp and Ln — one table set (natural_log_exp_and_others).

Layout: per-core shard of 256 samples; x host-transposed and blocked as
[128, 24, 256] so the contraction lands on SBUF partitions with
per-partition-contiguous DMA.  DMA issues come FIRST (before identity /
act warmup) on both HWDGE rings; W is split across the two rings so the
K = W^T W chain starts as early as possible, and x is loaded in 4
groups per ring (separate tiles -> precise per-group completion events,
the PE logits chain chases the DMA stream in arrival order; PSUM
accumulation order is commutative).  No SWDGE (gpsimd) DMAs: the K
DRAM-broadcast roundtrip rides the sync HWDGE ring, which also avoids
the Pool DGE-init memsets at kernel start.
"""

import numpy as np
from contextlib import ExitStack

import concourse.bass as bass
import concourse.bacc as bacc
import concourse.tile as tile
from concourse import mybir
from concourse.masks import make_identity

f32 = mybir.dt.float32
f16 = mybir.dt.float16
AF = mybir.ActivationFunctionType
OP = mybir.AluOpType
X = mybir.AxisListType.X

NCORES = 8
B = 2048
BL = B // NCORES          # 256 samples per core
D = 3072
C = 10
M = C - 1
P = 128
ND = D // P               # 24 contraction chunks
NG = BL // P              # 2 sample groups of 128
ALPHA = 1.0 - C * 1e-7
ISQ10 = 1.0 / float(np.sqrt(10.0))
# arccos series coeffs, folded with 2/EPSILON = 20
PC = [20.0 * c for c in (1.0, 1.0 / 12, 3.0 / 160, 5.0 / 896, 35.0 / 18432)]

# x chunk ranges per HWDGE ring (sync gets 0:12, scalar 12:24), small
# groups first so the PE chain starts early, larger later (descriptor
# overhead ~110ns/partition-line makes many small groups slow overall).
SYNC_CH = [(0, 2), (2, 5), (5, 8), (8, 12)]
SCAL_CH = [(12, 14), (14, 17), (17, 20), (20, 24)]
# PE consumption order: interleave rings by expected arrival.
MM_ORDER = [0, 4, 1, 5, 2, 6, 3, 7]   # index into SYNC_CH + SCAL_CH


def _v(t, dims, off=0):
    """Free-dim view of an SBUF tile AP, keeping its partition dim."""
    return bass.AP(
        tensor=t.tensor,
        offset=t.offset + off,
        ap=[list(t.ap[0])] + [list(d) for d in dims],
    )


def _patch_act_tables():
    """Force Exp/Ln to resolve to the one table set containing both, so the
    ACT table is loaded exactly once."""
    import concourse.hw_specs as hw_specs
    import concourse.bacc as bacc_mod
    if getattr(hw_specs, "_adaptive_temp_patched", False):
        return
    orig = hw_specs.get_activation_tables

    def patched(arch):
        t = orig(arch)
        for name, fns in t.items():
            if name != "natural_log_exp_and_others":
                fns.discard(AF.Exp)
                fns.discard(AF.Ln)
        return t

    hw_specs.get_activation_tables = patched
    hw_specs._adaptive_temp_patched = True
    for mod in (bacc_mod,):
        if hasattr(mod, "get_activation_tables"):
            mod.get_activation_tables = patched


def build_bass():
    _patch_act_tables()
    nc = bacc.Bacc("TRN2", target_bir_lowering=False, debug=False)
    # xh[k, i, s] = x[sample s, feature i*128+k]  (host pre-transposed)
    xh = nc.dram_tensor("xh", [P, ND, BL], f16, kind="ExternalInput").ap()
    # wh[k, i, c] = W[i*128+k, c]
    wh = nc.dram_tensor("wh", [P, ND, C], f16, kind="ExternalInput").ap()
    bt = nc.dram_tensor("bt", [C, 1], f32, kind="ExternalInput").ap()
    out = nc.dram_tensor("out", [BL, 1], f32, kind="ExternalOutput").ap()
    ksc = nc.dram_tensor("kscratch", [C, C], f32, kind="Internal").ap()

    with tile.TileContext(nc) as tc, ExitStack() as ctx:
        const = ctx.enter_context(tc.tile_pool(name="const", bufs=1))
        ps = ctx.enter_context(tc.tile_pool(name="ps", bufs=1, space="PSUM"))
        wk = ctx.enter_context(tc.tile_pool(name="wk", bufs=1))

        # ---- DMA issues first: W halves on both rings, then x groups.
        # btile rides the sync ring AFTER x (only needed ~10us later). ----
        wt = const.tile([P, ND, C], f16, name="wt")
        nc.sync.dma_start(out=wt[:, 0:12, :], in_=wh[:, 0:12, :])
        nc.scalar.dma_start(out=wt[:, 12:ND, :], in_=wh[:, 12:ND, :])

        xg = []
        for lo, hi in SYNC_CH:
            t = const.tile([P, hi - lo, BL], f16, name=f"xs{lo}")
            nc.sync.dma_start(out=t, in_=xh[:, lo:hi, :])
            xg.append((t, lo, hi))
        for lo, hi in SCAL_CH:
            t = const.tile([P, hi - lo, BL], f16, name=f"xa{lo}")
            nc.scalar.dma_start(out=t, in_=xh[:, lo:hi, :])
            xg.append((t, lo, hi))
        btile = const.tile([C, 1], f32, name="btile")
        nc.sync.dma_start(out=btile, in_=bt)

        # zero-bias APs created inside the tile context: float biases would
        # otherwise become const-pool memsets in the bass PREAMBLE, and those
        # pre-barrier gpsimd memsets start the profiler's exec-time clock
        # ~1.4us before the first DMA issue.
        zP = wk.tile([P, 1], f32, name="zP")
        nc.vector.memset(zP, 0.0)
        z1 = wk.tile([1, 1], f32, name="z1")
        nc.vector.memset(z1, 0.0)

        # ---- ACT table warmup (single natural_log_exp_and_others load) ----
        dum = const.tile([1, 1], f32, name="dum")
        nc.vector.memset(dum, 1.0)
        nc.scalar.activation(dum, dum, AF.Ln, bias=z1)
        nc.scalar.activation(dum, dum, AF.Exp, bias=z1)

        # ---- identity for PE transposes ----
        ident = const.tile([P, P], f32, name="ident")
        make_identity(nc, ident)

        # ---- K = W^T W (starts as soon as wt lands; x still streaming) ----
        kps = ps.tile([C, C], f32, name="kps")
        for i in range(ND):
            nc.tensor.matmul(kps, lhsT=wt[:, i, :], rhs=wt[:, i, :],
                             start=(i == 0), stop=(i == ND - 1))
        ksb = wk.tile([C, C], f32, name="ksb")
        nc.scalar.copy(ksb, kps)
        # broadcast K to all 128 partitions via a DRAM roundtrip (HWDGE)
        nc.sync.dma_start(out=ksc, in_=ksb)
        kb = const.tile([P, C, C], f32, name="kb")
        nc.sync.dma_start(
            out=kb,
            in_=bass.AP(tensor=ksc.tensor, offset=0, ap=[[0, P], [C, C], [1, C]]),
        )

        # 0.5*b for the exp(l/2) bias (b is tiny; Copy needs no table)
        bt05 = wk.tile([C, 1], f32, name="bt05")
        nc.scalar.mul(bt05, btile, 0.5)

        # ---- logits^T accumulation, chasing the DMA stream ----
        ltp = ps.tile([C, BL], f32, name="ltp")
        first = True
        n_done = 0
        for gi in MM_ORDER:
            t, lo, hi = xg[gi]
            for i in range(hi - lo):
                n_done += 1
                nc.tensor.matmul(ltp, lhsT=wt[:, lo + i, :], rhs=t[:, i, :],
                                 start=first, stop=(n_done == ND))
                first = False

        # ---- exp straight out of PSUM (bias folded), PE transposes ----
        et = wk.tile([C, BL], f32, name="et")
        nc.scalar.activation(out=et, in_=ltp, func=AF.Exp, bias=btile,
                             scale=1.0)
        e05 = wk.tile([C, BL], f32, name="e05")
        nc.scalar.activation(out=e05, in_=ltp, func=AF.Exp, bias=bt05,
                             scale=0.5)
        ug = ps.tile([P, NG, C], f32, name="ug")    # exp(l) sample-major
        vg = ps.tile([P, NG, C], f32, name="vg")    # exp(l/2) sample-major
        for g in range(NG):
            nc.tensor.transpose(ug[:, g, :], et[:, g * P:(g + 1) * P],
                                ident[0:C, 0:C])
        for g in range(NG):
            nc.tensor.transpose(vg[:, g, :], e05[:, g * P:(g + 1) * P],
                                ident[0:C, 0:C])

        # ---- nc = exp(l/2) * (alpha/S)^0.5 ----
        sumexp = wk.tile([P, NG], f32, name="sumexp")
        nc.vector.tensor_reduce(out=sumexp, in_=ug, axis=X, op=OP.add)
        lnS = wk.tile([P, NG], f32, name="lnS")
        nc.scalar.activation(out=lnS, in_=sumexp, func=AF.Ln, bias=zP,
                             scale=1.0 / ALPHA)
        rsqS = wk.tile([P, NG], f32, name="rsqS")
        nc.scalar.activation(out=rsqS, in_=lnS, func=AF.Exp, bias=zP,
                             scale=-0.5)
        ncv = wk.tile([P, NG, C], f32, name="ncv")
        rsqSe = _v(rsqS, [[1, NG], [0, C]])
        nc.vector.tensor_tensor(out=ncv, in0=vg, in1=rsqSe, op=OP.mult)
        sumnc = wk.tile([P, NG], f32, name="sumnc")
        nc.vector.tensor_reduce(out=sumnc, in_=ncv, axis=X, op=OP.add)

        ncM = _v(ncv, [[C, NG]], off=M)             # nc_9 per group  [P, 2]

        # ---- delta series (gpsimd + scalar: parallel to the DVE chain) ----
        e2 = wk.tile([P, NG], f32, name="e2")
        nc.gpsimd.tensor_scalar(out=e2, in0=sumnc, scalar1=-ISQ10,
                                scalar2=1.0, op0=OP.mult, op1=OP.add)
        ln2e = wk.tile([P, NG], f32, name="ln2e")
        nc.scalar.activation(out=ln2e, in_=e2, func=AF.Ln, bias=zP, scale=2.0)
        sq2e = wk.tile([P, NG], f32, name="sq2e")
        nc.scalar.activation(out=sq2e, in_=ln2e, func=AF.Exp, bias=zP,
                             scale=0.5)
        pol = wk.tile([P, NG], f32, name="pol")
        nc.gpsimd.tensor_scalar(out=pol, in0=e2, scalar1=PC[4], scalar2=PC[3],
                                op0=OP.mult, op1=OP.add)
        for k in (2, 1, 0):
            nc.gpsimd.tensor_mul(pol, pol, e2)
            nc.gpsimd.tensor_scalar_add(pol, pol, PC[k])
        num = wk.tile([P, NG], f32, name="num")
        nc.gpsimd.tensor_mul(num, sq2e, pol)

        # outer_{mn} = nc_m nc_n (gpsimd, off the DVE critical path)
        outer = wk.tile([P, NG, M, M], f32, name="outer")
        ncm_r = _v(ncv, [[C, NG], [1, M], [0, M]])
        ncm_c = _v(ncv, [[C, NG], [0, M], [1, M]])
        nc.gpsimd.tensor_tensor(out=outer, in0=ncm_r, in1=ncm_c, op=OP.mult)

        # ---- q = (p - nc9 e9)/(nc9 - 1): pt = nc^2 with col9 patched.
        # All on DVE: cross-engine ping-pong here costs ~200-300ns/hop. ----
        r1n = wk.tile([P, NG], f32, name="r1n")
        nc.vector.tensor_scalar(out=r1n, in0=ncM, scalar1=1.0, scalar2=None,
                                op0=OP.subtract)    # nc9 - 1
        g1n = wk.tile([P, NG], f32, name="g1n")
        nc.vector.reciprocal(g1n, r1n)
        pt = wk.tile([P, NG, C], f32, name="pt")
        nc.vector.tensor_mul(pt, ncv, ncv)          # p = nc^2
        ptM = _v(pt, [[C, NG]], off=M)
        nc.vector.tensor_tensor(out=ptM, in0=ncM, in1=r1n, op=OP.mult)
        q = wk.tile([P, NG, C], f32, name="q")
        g1ne = _v(g1n, [[1, NG], [0, C]])
        nc.vector.tensor_tensor(out=q, in0=pt, in1=g1ne, op=OP.mult)

        # ---- r = K q, c0 = q . r ----
        tmp = wk.tile([P, NG, C, C], f32, name="tmp")
        kb4 = _v(kb, [[0, NG], [C, C], [1, C]])
        q4 = _v(q, [[C, NG], [0, C], [1, C]])
        nc.vector.tensor_tensor(out=tmp, in0=kb4, in1=q4, op=OP.mult)
        r = wk.tile([P, NG, C], f32, name="r")
        nc.vector.tensor_reduce(out=r, in_=tmp, axis=X, op=OP.add)
        scr = wk.tile([P, NG, C], f32, name="scr")
        nc.vector.tensor_mul(scr, q, r)
        c0 = wk.tile([P, NG], f32, name="c0")
        nc.vector.tensor_reduce(out=c0, in_=scr, axis=X, op=OP.add)

        # ---- Gbar = (K[:9,:9] + r_m + r_n + c0) * outer; weighted norm ----
        gt = wk.tile([P, NG, M, M], f32, name="gt")
        r_rep = _v(r, [[C, NG], [1, M], [0, M]])
        r_til = _v(r, [[C, NG], [0, M], [1, M]])
        nc.vector.tensor_tensor(out=gt, in0=r_rep, in1=r_til, op=OP.add)
        kf4 = _v(kb, [[0, NG], [C, M], [1, M]])
        nc.vector.tensor_tensor(out=gt, in0=gt, in1=kf4, op=OP.add)
        c0e = _v(c0, [[1, NG], [0, M], [0, M]])
        nc.vector.tensor_tensor(out=gt, in0=gt, in1=c0e, op=OP.add)
        nc.vector.tensor_tensor(out=gt, in0=gt, in1=outer, op=OP.mult)
        rs = wk.tile([P, NG, M], f32, name="rs")
        nc.vector.tensor_reduce(out=rs, in_=gt, axis=X, op=OP.add,
                                apply_absolute_value=True)
        mx = wk.tile([P, NG], f32, name="mx")
        nc.vector.tensor_reduce(out=mx, in_=rs, axis=X, op=OP.max)
        rmx = wk.tile([P, NG], f32, name="rmx")
        nc.vector.reciprocal(rmx, mx)
        tempv = wk.tile([P, NG], f32, name="tempv")
        nc.vector.tensor_mul(tempv, num, rmx)

        # ---- transpose [128, 2] -> [2, 128]: contiguous output DMA ----
        otp = ps.tile([NG, P], f32, name="otp")
        nc.tensor.transpose(otp, tempv, ident)
        osb = wk.tile([NG, P], f32, name="osb")
        nc.vector.tensor_copy(osb, otp)
        nc.sync.dma_start(out=out.rearrange("(g p) o -> g (p o)", g=NG),
                          in_=osb)
    nc.compile()
    return nc


_NC_CACHE = None


def _get_nc():
    global _NC_CACHE
    if _NC_CACHE is None:
        _NC_CACHE = build_bass()
    return _NC_CACHE


def make_in_maps(data: np.ndarray, W: np.ndarray, b: np.ndarray):
    x = np.asarray(data, dtype=np.float32).reshape(B, D)
    Wf = np.ascontiguousarray(np.asarray(W, dtype=np.float32))
    btf = np.ascontiguousarray(np.asarray(b, dtype=np.float32).reshape(C, 1))
    whp = np.ascontiguousarray(
        Wf.reshape(ND, P, C).transpose(1, 0, 2).astype(np.float16))
    in_maps = []
    for i in range(NCORES):
        shard = x[i * BL:(i + 1) * BL, :]           # [256, 3072]
        xhp = np.ascontiguousarray(
            shard.T.reshape(ND, P, BL).transpose(1, 0, 2).astype(np.float16))
        in_maps.append({"xh": xhp, "wh": whp, "bt": btf})
    return in_maps


def kernel(data: np.ndarray, W: np.ndarray, b: np.ndarray) -> np.ndarray:
    from concourse.bass_utils import run_bass_kernel_spmd

    in_maps = make_in_maps(data, W, b)
    nc = _get_nc()
    res = run_bass_kernel_spmd(nc, in_maps, core_ids=list(range(NCORES)))
    outs = [res.results[i]["out"] for i in range(NCORES)]
    return np.concatenate(outs, axis=0).astype(np.float32)
